# revision 7
# baseline (speedup 1.0000x reference)
"""Trainium2 Bass kernel for CosineSim3D.

Reference computation (per batch element b):
    a_mag[n] = sqrt(max(sum_d A[n,d]^2, eps))
    b_mag[m] = sqrt(max(sum_d B[m,d]^2, eps))
    scores[n] = sum_m (A[n,:] . B[m,:]) / (a_mag[n] * b_mag[m])
    probs = softmax(scores)
    out[n, :] = probs[n]  (tiled 300x)

Key algebraic collapse: the [n,m] similarity matrix is never needed --
    scores[n] = (A[n,:] . c) / a_mag[n],   c[d] = sum_m B[m,d] / b_mag[m]
which turns an O(n*m*d) batched matmul into O(n*d) work, making the
kernel DMA-bound (each core streams its full input/output shard).

Sharding: pure data parallel over the batch dim, 128 batches -> 8 cores
x 16 batches each.  Full inputs in, full output out; shard/gather here.

Engine split per batch (all overlapped across batches by Tile):
  ScalarE: B squares (one big 3D op), A squares (8 accum ops), sqrt, exp
  VectorE: B square-reduce, dot mult+reduce (big 3D ops), small ops
  TensorE: partition reductions/broadcasts via tiny matmuls
  GpSimd:  probs -> [*, 300] expansion
  DMA:     ~3.7 MB/batch streaming (the bottleneck at ~358 GB/s/core)
"""

import numpy as np

import concourse.bacc as bacc
import concourse.bass as bass
import concourse.tile as tile
from concourse import mybir
from concourse.bass_utils import run_bass_kernel_spmd

# Problem shape (hardcoded per contract)
B_FULL = 128
N = 1024          # rows per batch (both a and b)
D = 300           # feature dim
N_CORES = 8
B_SHARD = B_FULL // N_CORES   # 16 batches per core
P = 128           # SBUF partitions
C = N // P        # 8 row-chunks of 128 per batch
EPS = 1e-7

F32 = mybir.dt.float32
AF = mybir.ActivationFunctionType
ALU = mybir.AluOpType
AX = mybir.AxisListType


def _build_program() -> bass.Bass:
    nc = bacc.Bacc(
        "TRN2",
        target_bir_lowering=False,
        debug=False,
        num_devices=N_CORES,
    )

    a_h = nc.declare_dram_parameter("a", [B_SHARD, N, D], F32, isOutput=False)
    b_h = nc.declare_dram_parameter("b", [B_SHARD, N, D], F32, isOutput=False)
    o_h = nc.declare_dram_parameter("out", [B_SHARD, N, D], F32, isOutput=True)

    # row index = p*C + c  ->  each partition holds C contiguous rows (9600 B)
    a_v = a_h[:].rearrange("s (p c) d -> s p c d", p=P)
    b_v = b_h[:].rearrange("s (p c) d -> s p c d", p=P)
    o_v = o_h[:].rearrange("s (p c) d -> s p c d", p=P)

    with tile.TileContext(nc) as tc:
        with (
            tc.tile_pool(name="singles", bufs=1) as singles,
            tc.tile_pool(name="big", bufs=3) as big,
            tc.tile_pool(name="mid", bufs=2) as mid,
            tc.tile_pool(name="small", bufs=3) as small,
            tc.tile_pool(name="psum", bufs=2, space="PSUM") as psum,
        ):
            ones_wide = singles.tile([P, D], F32, tag="ones_wide")
            nc.vector.memset(ones_wide, 1.0)
            ones_row = singles.tile([1, P], F32, tag="ones_row")
            nc.vector.memset(ones_row, 1.0)
            ones_col = singles.tile([P, 1], F32, tag="ones_col")
            nc.vector.memset(ones_col, 1.0)

            for i in range(B_SHARD):
                # ---- load shard batch i ----
                b_tile = big.tile([P, C, D], F32, tag="b_tile")
                nc.sync.dma_start(out=b_tile, in_=b_v[i])
                a_tile = big.tile([P, C, D], F32, tag="a_tile")
                nc.sync.dma_start(out=a_tile, in_=a_v[i])

                # ss_ab[:, :C] = per-row sum(B^2); ss_ab[:, C:] = sum(A^2)
                ss_ab = small.tile([P, 2 * C], F32, tag="ss_ab")

                # B squares: one big ACT op, then one big DVE 3D reduce
                bsq = mid.tile([P, C, D], F32, tag="bsq")
                nc.scalar.activation(out=bsq, in_=b_tile, func=AF.Square)
                nc.vector.tensor_reduce(
                    out=ss_ab[:, 0:C], in_=bsq, axis=AX.X, op=ALU.add
                )

                # A squares: ACT square + horizontal accumulate per chunk
                sq_scr = mid.tile([P, D], F32, tag="sq_scr")
                for j in range(C):
                    nc.scalar.activation(
                        out=sq_scr,
                        in_=a_tile[:, j, :],
                        func=AF.Square,
                        accum_out=ss_ab[:, C + j : C + j + 1],
                    )

                # inv = 1/sqrt(max(ss, eps)); [:, :C] = binv, [:, C:] = ainv
                inv = small.tile([P, 2 * C], F32, tag="inv")
                nc.vector.tensor_scalar_max(out=inv, in0=ss_ab, scalar1=EPS)
                nc.scalar.activation(out=inv, in_=inv, func=AF.Sqrt)
                nc.vector.reciprocal(out=inv, in_=inv)

                # ---- c[d] = sum_m B[m,d] * binv[m]  (PE partition-reduce) ----
                c_ps = psum.tile([1, D], F32, tag="c_ps")
                for j in range(C):
                    nc.tensor.matmul(
                        c_ps,
                        inv[:, j : j + 1],       # lhsT [K=128, M=1]
                        b_tile[:, j, :],         # rhs  [K=128, N=300]
                        start=(j == 0),
                        stop=(j == C - 1),
                    )
                c_sb = small.tile([1, D], F32, tag="c_sb")
                nc.scalar.copy(c_sb, c_ps)

                # broadcast c across partitions: ones[1(K),128] x c[1(K),300]
                cb_ps = psum.tile([P, D], F32, tag="cb_ps")
                nc.tensor.matmul(cb_ps, ones_row, c_sb, start=True, stop=True)

                # ---- dot[n] = A[n,:] . c  (big DVE mult + 3D reduce) ----
                prod = mid.tile([P, C, D], F32, tag="prod")
                nc.vector.tensor_mul(
                    prod, a_tile, cb_ps.unsqueeze(1).broadcast_to([P, C, D])
                )
                dot = small.tile([P, C], F32, tag="dot")
                nc.vector.tensor_reduce(out=dot, in_=prod, axis=AX.X, op=ALU.add)

                # scores = dot * ainv ; exp + per-partition row sums
                scores = small.tile([P, C], F32, tag="scores")
                nc.vector.tensor_mul(scores, dot, inv[:, C : 2 * C])
                exp_s = small.tile([P, C], F32, tag="exp_s")
                row_sum = small.tile([P, 1], F32, tag="row_sum")
                nc.scalar.activation(
                    out=exp_s, in_=scores, func=AF.Exp, accum_out=row_sum
                )

                # Z = sum over partitions; invZ broadcast back to all rows
                z_ps = psum.tile([1, 1], F32, tag="z_ps")
                nc.tensor.matmul(z_ps, row_sum, ones_col, start=True, stop=True)
                inv_z = small.tile([1, 1], F32, tag="inv_z")
                nc.vector.reciprocal(out=inv_z, in_=z_ps)
                invz_ps = psum.tile([P, 1], F32, tag="invz_ps")
                nc.tensor.matmul(invz_ps, ones_row, inv_z, start=True, stop=True)

                probs = small.tile([P, C], F32, tag="probs")
                nc.vector.tensor_mul(
                    probs, exp_s, invz_ps.broadcast_to([P, C])
                )

                # ---- expand probs -> [P, C, 300] on GpSimd and store ----
                out_tile = big.tile([P, C, D], F32, tag="out_tile")
                for j in range(C):
                    nc.gpsimd.tensor_scalar_mul(
                        out=out_tile[:, j, :],
                        in0=ones_wide,
                        scalar1=probs[:, j : j + 1],
                    )
                nc.sync.dma_start(out=o_v[i], in_=out_tile)

    nc.finalize()
    return nc


_NC_CACHE = None


def _get_program():
    global _NC_CACHE
    if _NC_CACHE is None:
        _NC_CACHE = _build_program()
    return _NC_CACHE


def run(a: np.ndarray, b: np.ndarray, trace: bool = False):
    """Shard over batch, run on 8 cores, gather. Returns (out, BassKernelResults)."""
    a = np.ascontiguousarray(a, dtype=np.float32)
    b = np.ascontiguousarray(b, dtype=np.float32)
    assert a.shape == (B_FULL, N, D) and b.shape == (B_FULL, N, D)

    nc = _get_program()
    in_maps = [
        {
            "a": a[i * B_SHARD : (i + 1) * B_SHARD],
            "b": b[i * B_SHARD : (i + 1) * B_SHARD],
        }
        for i in range(N_CORES)
    ]
    res = run_bass_kernel_spmd(nc, in_maps, list(range(N_CORES)), trace=trace)
    out = np.concatenate([r["out"] for r in res.results], axis=0)
    return out, res


def kernel(a: np.ndarray, b: np.ndarray) -> np.ndarray:
    out, _ = run(a, b, trace=False)
    return out


# revision 9
# speedup vs baseline: 2.7645x; 2.7645x over previous
"""Trainium2 Bass kernel for CosineSim3D.

Reference computation (per batch element b):
    a_mag[n] = sqrt(max(sum_d A[n,d]^2, eps))
    b_mag[m] = sqrt(max(sum_d B[m,d]^2, eps))
    scores[n] = sum_m (A[n,:] . B[m,:]) / (a_mag[n] * b_mag[m])
    probs = softmax(scores)
    out[n, :] = probs[n]  (tiled 300x)

Key algebraic collapse: the [n,m] similarity matrix is never needed --
    scores[n] = (A[n,:] . c) / a_mag[n],   c[d] = sum_m B[m,d] / b_mag[m]
which turns an O(n*m*d) batched matmul into O(n*d) work, making the
kernel DMA-bound (each core streams its full input/output shard).

Sharding: pure data parallel over the batch dim, 128 batches -> 8 cores
x 16 batches each.  Full inputs in, full output out; shard/gather here.

Engine split per batch (all overlapped across batches by Tile):
  ScalarE: B squares (one big 3D op), A squares (8 accum ops), sqrt, exp
  VectorE: B square-reduce, dot mult+reduce (big 3D ops), small ops
  TensorE: partition reductions/broadcasts via tiny matmuls
  GpSimd:  probs -> [*, 300] expansion
  DMA:     ~3.7 MB/batch streaming (the bottleneck at ~358 GB/s/core)
"""

import numpy as np

import concourse.bacc as bacc
import concourse.bass as bass
import concourse.tile as tile
from concourse import mybir
from concourse.bass_utils import run_bass_kernel_spmd

# Problem shape (hardcoded per contract)
B_FULL = 128
N = 1024          # rows per batch (both a and b)
D = 300           # feature dim
N_CORES = 8
B_SHARD = B_FULL // N_CORES   # 16 batches per core
P = 128           # SBUF partitions
C = N // P        # 8 row-chunks of 128 per batch
EPS = 1e-7

F32 = mybir.dt.float32
AF = mybir.ActivationFunctionType
ALU = mybir.AluOpType
AX = mybir.AxisListType


def _build_program() -> bass.Bass:
    nc = bacc.Bacc(
        "TRN2",
        target_bir_lowering=False,
        debug=False,
        num_devices=N_CORES,
    )

    a_h = nc.declare_dram_parameter("a", [B_SHARD, N, D], F32, isOutput=False)
    b_h = nc.declare_dram_parameter("b", [B_SHARD, N, D], F32, isOutput=False)
    o_h = nc.declare_dram_parameter("out", [B_SHARD, N, D], F32, isOutput=True)

    # row index = p*C + c  ->  each partition holds C contiguous rows (9600 B)
    a_v = a_h[:].rearrange("s (p c) d -> s p c d", p=P)
    b_v = b_h[:].rearrange("s (p c) d -> s p c d", p=P)
    o_v = o_h[:].rearrange("s (p c) d -> s p c d", p=P)

    with tile.TileContext(nc) as tc:
        with (
            tc.tile_pool(name="singles", bufs=1) as singles,
            tc.tile_pool(name="big", bufs=3) as big,
            tc.tile_pool(name="mid", bufs=2) as mid,
            tc.tile_pool(name="small", bufs=3) as small,
            tc.tile_pool(name="psum", bufs=2, space="PSUM") as psum,
        ):
            ones_wide = singles.tile([P, D], F32, tag="ones_wide")
            nc.vector.memset(ones_wide, 1.0)
            ones_row = singles.tile([1, P], F32, tag="ones_row")
            nc.vector.memset(ones_row, 1.0)
            ones_col = singles.tile([P, 1], F32, tag="ones_col")
            nc.vector.memset(ones_col, 1.0)

            for i in range(B_SHARD):
                # ---- load shard batch i ----
                b_tile = big.tile([P, C, D], F32, tag="b_tile")
                nc.sync.dma_start(out=b_tile, in_=b_v[i])
                a_tile = big.tile([P, C, D], F32, tag="a_tile")
                nc.sync.dma_start(out=a_tile, in_=a_v[i])

                # inv holds 1/sqrt(max(ss,eps)): [:, :C] = binv, [:, C:] = ainv
                inv = small.tile([P, 2 * C], F32, tag="inv")

                # B row sums of squares via bn_stats (one DVE pass per chunk):
                # ss = (var + mean^2) * D
                bnst = small.tile([P, C, 6], F32, tag="bnst")
                mv = small.tile([P, C, 2], F32, tag="mv")
                for j in range(C):
                    nc.vector.bn_stats(out=bnst[:, j, :], in_=b_tile[:, j, :])
                    nc.vector.bn_aggr(out=mv[:, j, :], in_=bnst[:, j, :])
                m2 = small.tile([P, C], F32, tag="m2")
                nc.vector.tensor_mul(m2, mv[:, :, 0], mv[:, :, 0])
                nc.vector.tensor_add(m2, m2, mv[:, :, 1])
                nc.vector.tensor_scalar(
                    out=inv[:, 0:C],
                    in0=m2,
                    scalar1=float(D),
                    scalar2=EPS,
                    op0=ALU.mult,
                    op1=ALU.max,
                )

                # A squares: ACT square + horizontal accumulate per chunk
                ssa = small.tile([P, C], F32, tag="ssa")
                sq_scr = mid.tile([P, D], F32, tag="sq_scr")
                for j in range(C):
                    nc.scalar.activation(
                        out=sq_scr,
                        in_=a_tile[:, j, :],
                        func=AF.Square,
                        accum_out=ssa[:, j : j + 1],
                    )
                nc.vector.tensor_scalar_max(
                    out=inv[:, C : 2 * C], in0=ssa, scalar1=EPS
                )

                nc.scalar.activation(out=inv, in_=inv, func=AF.Sqrt)
                nc.vector.reciprocal(out=inv, in_=inv)

                # ---- c[d] = sum_m B[m,d] * binv[m]  (PE partition-reduce) ----
                c_ps = psum.tile([1, D], F32, tag="c_ps")
                for j in range(C):
                    nc.tensor.matmul(
                        c_ps,
                        inv[:, j : j + 1],       # lhsT [K=128, M=1]
                        b_tile[:, j, :],         # rhs  [K=128, N=300]
                        start=(j == 0),
                        stop=(j == C - 1),
                    )
                c_sb = small.tile([1, D], F32, tag="c_sb")
                nc.scalar.copy(c_sb, c_ps)

                # broadcast c across partitions: ones[1(K),128] x c[1(K),300]
                cb_ps = psum.tile([P, D], F32, tag="cb_ps")
                nc.tensor.matmul(cb_ps, ones_row, c_sb, start=True, stop=True)

                # ---- dot[n] = A[n,:] . c  (big DVE mult + 3D reduce) ----
                prod = mid.tile([P, C, D], F32, tag="prod")
                nc.vector.tensor_mul(
                    prod, a_tile, cb_ps.unsqueeze(1).broadcast_to([P, C, D])
                )
                dot = small.tile([P, C], F32, tag="dot")
                nc.vector.tensor_reduce(out=dot, in_=prod, axis=AX.X, op=ALU.add)

                # scores = dot * ainv ; exp + per-partition row sums
                scores = small.tile([P, C], F32, tag="scores")
                nc.vector.tensor_mul(scores, dot, inv[:, C : 2 * C])
                exp_s = small.tile([P, C], F32, tag="exp_s")
                row_sum = small.tile([P, 1], F32, tag="row_sum")
                nc.scalar.activation(
                    out=exp_s, in_=scores, func=AF.Exp, accum_out=row_sum
                )

                # Z = sum over partitions; invZ broadcast back to all rows
                z_ps = psum.tile([1, 1], F32, tag="z_ps")
                nc.tensor.matmul(z_ps, row_sum, ones_col, start=True, stop=True)
                inv_z = small.tile([1, 1], F32, tag="inv_z")
                nc.vector.reciprocal(out=inv_z, in_=z_ps)
                invz_ps = psum.tile([P, 1], F32, tag="invz_ps")
                nc.tensor.matmul(invz_ps, ones_row, inv_z, start=True, stop=True)

                probs = small.tile([P, C], F32, tag="probs")
                nc.vector.tensor_mul(
                    probs, exp_s, invz_ps.broadcast_to([P, C])
                )

                # ---- expand probs -> [P, C, 300], split ACT/DVE, store ----
                out_tile = big.tile([P, C, D], F32, tag="out_tile")
                for j in range(C):
                    if j % 2 == 0:
                        nc.scalar.activation(
                            out=out_tile[:, j, :],
                            in_=ones_wide,
                            func=AF.Copy,
                            scale=probs[:, j : j + 1],
                        )
                    else:
                        nc.vector.tensor_scalar_mul(
                            out=out_tile[:, j, :],
                            in0=ones_wide,
                            scalar1=probs[:, j : j + 1],
                        )
                nc.sync.dma_start(out=o_v[i], in_=out_tile)

    nc.finalize()
    return nc


_NC_CACHE = None


def _get_program():
    global _NC_CACHE
    if _NC_CACHE is None:
        _NC_CACHE = _build_program()
    return _NC_CACHE


def run(a: np.ndarray, b: np.ndarray, trace: bool = False):
    """Shard over batch, run on 8 cores, gather. Returns (out, BassKernelResults)."""
    a = np.ascontiguousarray(a, dtype=np.float32)
    b = np.ascontiguousarray(b, dtype=np.float32)
    assert a.shape == (B_FULL, N, D) and b.shape == (B_FULL, N, D)

    nc = _get_program()
    in_maps = [
        {
            "a": a[i * B_SHARD : (i + 1) * B_SHARD],
            "b": b[i * B_SHARD : (i + 1) * B_SHARD],
        }
        for i in range(N_CORES)
    ]
    res = run_bass_kernel_spmd(nc, in_maps, list(range(N_CORES)), trace=trace)
    out = np.concatenate([r["out"] for r in res.results], axis=0)
    return out, res


def kernel(a: np.ndarray, b: np.ndarray) -> np.ndarray:
    out, _ = run(a, b, trace=False)
    return out


# revision 10
# speedup vs baseline: 2.8462x; 1.0296x over previous
"""Trainium2 Bass kernel for CosineSim3D.

Reference computation (per batch element b):
    a_mag[n] = sqrt(max(sum_d A[n,d]^2, eps))
    b_mag[m] = sqrt(max(sum_d B[m,d]^2, eps))
    scores[n] = sum_m (A[n,:] . B[m,:]) / (a_mag[n] * b_mag[m])
    probs = softmax(scores)
    out[n, :] = probs[n]  (tiled 300x)

Key algebraic collapse: the [n,m] similarity matrix is never needed --
    scores[n] = (A[n,:] . c) / a_mag[n],   c[d] = sum_m B[m,d] / b_mag[m]
which turns an O(n*m*d) batched matmul into O(n*d) work, making the
kernel DMA-bound (each core streams its full input/output shard).

Sharding: pure data parallel over the batch dim, 128 batches -> 8 cores
x 16 batches each.  Full inputs in, full output out; shard/gather here.

Engine split per batch (all overlapped across batches by Tile):
  ScalarE: B squares (one big 3D op), A squares (8 accum ops), sqrt, exp
  VectorE: B square-reduce, dot mult+reduce (big 3D ops), small ops
  TensorE: partition reductions/broadcasts via tiny matmuls
  GpSimd:  probs -> [*, 300] expansion
  DMA:     ~3.7 MB/batch streaming (the bottleneck at ~358 GB/s/core)
"""

import numpy as np

import concourse.bacc as bacc
import concourse.bass as bass
import concourse.tile as tile
from concourse import mybir
from concourse.bass_utils import run_bass_kernel_spmd

# Problem shape (hardcoded per contract)
B_FULL = 128
N = 1024          # rows per batch (both a and b)
D = 300           # feature dim
N_CORES = 8
B_SHARD = B_FULL // N_CORES   # 16 batches per core
P = 128           # SBUF partitions
C = N // P        # 8 row-chunks of 128 per batch
EPS = 1e-7

F32 = mybir.dt.float32
AF = mybir.ActivationFunctionType
ALU = mybir.AluOpType
AX = mybir.AxisListType


def _build_program() -> bass.Bass:
    nc = bacc.Bacc(
        "TRN2",
        target_bir_lowering=False,
        debug=False,
        num_devices=N_CORES,
    )

    a_h = nc.declare_dram_parameter("a", [B_SHARD, N, D], F32, isOutput=False)
    b_h = nc.declare_dram_parameter("b", [B_SHARD, N, D], F32, isOutput=False)
    o_h = nc.declare_dram_parameter("out", [B_SHARD, N, D], F32, isOutput=True)

    # row index = p*C + c  ->  each partition holds C contiguous rows (9600 B)
    a_v = a_h[:].rearrange("s (p c) d -> s p c d", p=P)
    b_v = b_h[:].rearrange("s (p c) d -> s p c d", p=P)
    o_v = o_h[:].rearrange("s (p c) d -> s p c d", p=P)

    with tile.TileContext(nc) as tc:
        with (
            tc.tile_pool(name="singles", bufs=1) as singles,
            tc.tile_pool(name="big", bufs=4) as big,
            tc.tile_pool(name="mid", bufs=3) as mid,
            tc.tile_pool(name="small", bufs=6) as small,
            tc.tile_pool(name="psum", bufs=2, space="PSUM") as psum,
        ):
            ones_wide = singles.tile([P, D], F32, tag="ones_wide")
            nc.vector.memset(ones_wide, 1.0)
            ones_row = singles.tile([1, P], F32, tag="ones_row")
            nc.vector.memset(ones_row, 1.0)
            ones_col = singles.tile([P, 1], F32, tag="ones_col")
            nc.vector.memset(ones_col, 1.0)

            for i in range(B_SHARD):
                # ---- load shard batch i ----
                b_tile = big.tile([P, C, D], F32, tag="b_tile")
                nc.sync.dma_start(out=b_tile, in_=b_v[i])
                a_tile = big.tile([P, C, D], F32, tag="a_tile")
                nc.sync.dma_start(out=a_tile, in_=a_v[i])

                # inv holds 1/sqrt(max(ss,eps)): [:, :C] = binv, [:, C:] = ainv
                inv = small.tile([P, 2 * C], F32, tag="inv")

                # B row sums of squares via bn_stats (one DVE pass per chunk):
                # ss = (var + mean^2) * D
                bnst = small.tile([P, C, 6], F32, tag="bnst")
                mv = small.tile([P, C, 2], F32, tag="mv")
                for j in range(C):
                    nc.vector.bn_stats(out=bnst[:, j, :], in_=b_tile[:, j, :])
                    nc.vector.bn_aggr(out=mv[:, j, :], in_=bnst[:, j, :])
                m2 = small.tile([P, C], F32, tag="m2")
                nc.vector.tensor_mul(m2, mv[:, :, 0], mv[:, :, 0])
                nc.vector.tensor_add(m2, m2, mv[:, :, 1])
                nc.vector.tensor_scalar(
                    out=inv[:, 0:C],
                    in0=m2,
                    scalar1=float(D),
                    scalar2=EPS,
                    op0=ALU.mult,
                    op1=ALU.max,
                )

                # A squares: ACT square + horizontal accumulate per chunk
                ssa = small.tile([P, C], F32, tag="ssa")
                sq_scr = mid.tile([P, D], F32, tag="sq_scr")
                for j in range(C):
                    nc.scalar.activation(
                        out=sq_scr,
                        in_=a_tile[:, j, :],
                        func=AF.Square,
                        accum_out=ssa[:, j : j + 1],
                    )
                nc.vector.tensor_scalar_max(
                    out=inv[:, C : 2 * C], in0=ssa, scalar1=EPS
                )

                nc.scalar.activation(out=inv, in_=inv, func=AF.Sqrt)
                nc.vector.reciprocal(out=inv, in_=inv)

                # ---- c[d] = sum_m B[m,d] * binv[m]  (PE partition-reduce) ----
                c_ps = psum.tile([1, D], F32, tag="c_ps")
                for j in range(C):
                    nc.tensor.matmul(
                        c_ps,
                        inv[:, j : j + 1],       # lhsT [K=128, M=1]
                        b_tile[:, j, :],         # rhs  [K=128, N=300]
                        start=(j == 0),
                        stop=(j == C - 1),
                    )
                c_sb = small.tile([1, D], F32, tag="c_sb")
                nc.scalar.copy(c_sb, c_ps)

                # broadcast c across partitions: ones[1(K),128] x c[1(K),300]
                cb_ps = psum.tile([P, D], F32, tag="cb_ps")
                nc.tensor.matmul(cb_ps, ones_row, c_sb, start=True, stop=True)

                # ---- dot[n] = A[n,:] . c  (big DVE mult + 3D reduce) ----
                prod = mid.tile([P, C, D], F32, tag="prod")
                nc.vector.tensor_mul(
                    prod, a_tile, cb_ps.unsqueeze(1).broadcast_to([P, C, D])
                )
                dot = small.tile([P, C], F32, tag="dot")
                nc.vector.tensor_reduce(out=dot, in_=prod, axis=AX.X, op=ALU.add)

                # scores = dot * ainv ; exp + per-partition row sums
                scores = small.tile([P, C], F32, tag="scores")
                nc.vector.tensor_mul(scores, dot, inv[:, C : 2 * C])
                exp_s = small.tile([P, C], F32, tag="exp_s")
                row_sum = small.tile([P, 1], F32, tag="row_sum")
                nc.scalar.activation(
                    out=exp_s, in_=scores, func=AF.Exp, accum_out=row_sum
                )

                # Z = sum over partitions; invZ broadcast back to all rows
                z_ps = psum.tile([1, 1], F32, tag="z_ps")
                nc.tensor.matmul(z_ps, row_sum, ones_col, start=True, stop=True)
                inv_z = small.tile([1, 1], F32, tag="inv_z")
                nc.vector.reciprocal(out=inv_z, in_=z_ps)
                invz_ps = psum.tile([P, 1], F32, tag="invz_ps")
                nc.tensor.matmul(invz_ps, ones_row, inv_z, start=True, stop=True)

                probs = small.tile([P, C], F32, tag="probs")
                nc.vector.tensor_mul(
                    probs, exp_s, invz_ps.broadcast_to([P, C])
                )

                # ---- expand probs -> [P, C, 300], split ACT/DVE, store ----
                out_tile = big.tile([P, C, D], F32, tag="out_tile")
                for j in range(C):
                    if j % 2 == 0:
                        nc.scalar.activation(
                            out=out_tile[:, j, :],
                            in_=ones_wide,
                            func=AF.Copy,
                            scale=probs[:, j : j + 1],
                        )
                    else:
                        nc.vector.tensor_scalar_mul(
                            out=out_tile[:, j, :],
                            in0=ones_wide,
                            scalar1=probs[:, j : j + 1],
                        )
                nc.sync.dma_start(out=o_v[i], in_=out_tile)

    nc.finalize()
    return nc


_NC_CACHE = None


def _get_program():
    global _NC_CACHE
    if _NC_CACHE is None:
        _NC_CACHE = _build_program()
    return _NC_CACHE


def run(a: np.ndarray, b: np.ndarray, trace: bool = False):
    """Shard over batch, run on 8 cores, gather. Returns (out, BassKernelResults)."""
    a = np.ascontiguousarray(a, dtype=np.float32)
    b = np.ascontiguousarray(b, dtype=np.float32)
    assert a.shape == (B_FULL, N, D) and b.shape == (B_FULL, N, D)

    nc = _get_program()
    in_maps = [
        {
            "a": a[i * B_SHARD : (i + 1) * B_SHARD],
            "b": b[i * B_SHARD : (i + 1) * B_SHARD],
        }
        for i in range(N_CORES)
    ]
    res = run_bass_kernel_spmd(nc, in_maps, list(range(N_CORES)), trace=trace)
    out = np.concatenate([r["out"] for r in res.results], axis=0)
    return out, res


def kernel(a: np.ndarray, b: np.ndarray) -> np.ndarray:
    out, _ = run(a, b, trace=False)
    return out
